# revision 2
# baseline (speedup 1.0000x reference)
"""Trainium2 Bass kernel for CombinedRepeatCausalLinear (parallel forward).

Computes out[b,e,t] = sum_s x[b,e,s] * W[s,t] + bias[t] where
  W[s,t] = mask(t>=s) * (w0[s]*d0^(t-s) + w1[t]*d1^(t-s))
for S = 2048, x of shape (8, 1024, 2048) fp32.

Strategy (8 NeuronCores, data-parallel over batch):
  - core c gets x[c] (1024 rows); host pre-transposes to xT (2048, 1024) so
    the contraction dim lands on SBUF partitions with contiguous DMAs.
  - W is rank-2 before causal masking: each (128 s) x (512 t) chunk of W is
    generated ON-CHIP by a K=2 matmul from tiny host-precomputed factor
    vectors (per-chunk exponent offsets keep fp32 in range), then boundary
    chunks are multiplied by one of 4 precomputed 0/1 causal masks on DVE.
  - main matmul runs in float32r (full-rate fp32 mode, 1 cyc/row at N=512):
    outT[t,r] = sum_s W[s,t] * xT[s,r], accumulated over s-tiles in PSUM,
    skipping all-zero below-diagonal blocks (272 of 512 matmuls).
  - bias is fused into the PSUM->SBUF copy on the scalar engine
    (activation Identity with per-partition bias).
  - host transposes each core's outT back and stacks.
"""

import numpy as np

import concourse.bass as bass
import concourse.mybir as mybir
import concourse.tile as tile
from concourse import bacc
from concourse.bass_utils import run_bass_kernel_spmd

F32 = mybir.dt.float32
F32R = mybir.dt.float32r

B = 8
E = 1024
S = 2048
DC = 1.0
N_CORES = 8
R = (B * E) // N_CORES      # rows per core = 1024
ST = S // 128               # 16 s-tiles of 128
TB = S // 512               # 4 t-blocks of 512
RB = R // 512               # 2 r-blocks of 512

# chunk list: (si, tb) with si <= 4*tb+3  (40 chunks)
CHUNKS = [(si, tb) for tb in range(TB) for si in range(min(ST, 4 * tb + 4))]
CHUNK_IDX = {c: i for i, c in enumerate(CHUNKS)}
N_CHUNKS = len(CHUNKS)

_PROGRAM = None  # (nc, ...) cache


def _build_program(repeats=1, no_wgen=False, no_store=False, no_xload=False,
                   po_bufs=4, wc_bufs=30, osb_bufs=4, xsplit=1):
    nc = bacc.Bacc("TRN2", target_bir_lowering=False, debug=False,
                   num_devices=N_CORES)

    xT_d = nc.declare_dram_parameter("xT", [S, R], F32, isOutput=False)
    wstat_d = nc.declare_dram_parameter("wstat", [N_CHUNKS, 2, 128], F32,
                                        isOutput=False)
    wmov_d = nc.declare_dram_parameter("wmov", [N_CHUNKS, 2, 512], F32,
                                       isOutput=False)
    masks_d = nc.declare_dram_parameter("masks", [4, 128, 512], F32,
                                        isOutput=False)
    biasT_d = nc.declare_dram_parameter("biasT", [128, ST], F32,
                                        isOutput=False)
    outT_d = nc.declare_dram_parameter("outT", [S, R], F32, isOutput=True)

    with tile.TileContext(nc) as tc:
        with (
            tc.tile_pool(name="xp", bufs=1) as xp,
            tc.tile_pool(name="cst", bufs=1) as cst,
            tc.tile_pool(name="wg", bufs=6) as wg,
            tc.tile_pool(name="wc", bufs=wc_bufs) as wcp,
            tc.tile_pool(name="osb", bufs=osb_bufs) as osb,
            tc.tile_pool(name="pw", bufs=2, space="PSUM") as pwp,
            tc.tile_pool(name="po", bufs=po_bufs, space="PSUM") as pop,
        ):
            mask_sb = []
            for m in range(4):
                mt = cst.tile([128, 512], F32, tag=f"mask{m}")
                nc.gpsimd.dma_start(mt[:], masks_d[m])
                mask_sb.append(mt)
            bias_sb = cst.tile([128, ST], F32, tag="bias")
            nc.gpsimd.dma_start(bias_sb[:], biasT_d[:])

            for rep in range(repeats):
              # resident x tiles: [128 s, 1024 r] per s-tile
              xs = []
              for si in range(ST):
                t = xp.tile([128, R], F32R, tag=f"x{si}", name=f"x{si}_{rep}")
                if not no_xload:
                    for xs_i in range(xsplit):
                        w0c = (R // xsplit) * xs_i
                        w1c = (R // xsplit) * (xs_i + 1)
                        nc.sync.dma_start(
                            t[:, w0c:w1c],
                            xT_d[128 * si:128 * (si + 1), w0c:w1c]
                            .bitcast(F32R))
                xs.append(t)
              def emit_wgen(tb):
                # generate W chunks (si, tb) for t-block tb
                w_sb = []
                for si in range(min(ST, 4 * tb + 4)):
                    w = wcp.tile([128, 512], F32R, tag="wc", name=f"w{tb}_{si}")
                    if no_wgen:
                        nc.gpsimd.memset(w[:], 0.0)
                    else:
                        ci = CHUNK_IDX[(si, tb)]
                        st = wg.tile([2, 128], F32R, tag="wstat", name="st")
                        nc.gpsimd.dma_start(st[:], wstat_d[ci].bitcast(F32R))
                        mv = wg.tile([2, 512], F32R, tag="wmov", name="mv")
                        nc.gpsimd.dma_start(mv[:], wmov_d[ci].bitcast(F32R))
                        psw = pwp.tile([128, 512], F32, tag="pw", name="psw")
                        nc.tensor.matmul(psw[:], st[:], mv[:], start=True,
                                         stop=True)
                        d2 = 4 * tb - si
                        if d2 <= 0:
                            nc.vector.tensor_mul(w[:], psw[:],
                                                 mask_sb[d2 + 3][:])
                        else:
                            nc.vector.tensor_copy(w[:], psw[:])
                    w_sb.append(w)
                return w_sb

              w_by_tb = {0: emit_wgen(0), 1: emit_wgen(1)}
              for tb in range(TB):
                w_sb = w_by_tb.pop(tb)
                for tjl in range(4):
                    tj = 4 * tb + tjl
                    out_sb = osb.tile([128, R], F32, tag="osb")
                    ps = [pop.tile([128, 512], F32, tag="po", name=f"po{rb}")
                          for rb in range(RB)]
                    for si in range(tj + 1):
                        lhsT = w_sb[si][:, 128 * tjl:128 * (tjl + 1)]
                        for rb in range(RB):
                            nc.tensor.matmul(
                                ps[rb][:], lhsT,
                                xs[si][:, 512 * rb:512 * (rb + 1)],
                                start=(si == 0), stop=(si == tj),
                            )
                    for rb in range(RB):
                        nc.scalar.activation(
                            out_sb[:, 512 * rb:512 * (rb + 1)], ps[rb][:],
                            mybir.ActivationFunctionType.Identity,
                            bias=bias_sb[:, tj:tj + 1],
                        )
                    if not no_store:
                        nc.sync.dma_start(
                            outT_d[128 * tj:128 * (tj + 1), :], out_sb[:])
                if tb + 2 < TB:
                    w_by_tb[tb + 2] = emit_wgen(tb + 2)

    nc.compile()
    return nc


def _host_prep(weight, bias, decay_value):
    w0 = weight[0].astype(np.float64)
    w1 = weight[1].astype(np.float64)
    d0 = float(np.clip(np.float32(decay_value[0, 0]), 0.9, 1.0))
    d1 = float(np.clip(np.float32(decay_value[1, 0]), 0.9, 1.0))
    ii = np.arange(128, dtype=np.float64)
    jj = np.arange(512, dtype=np.float64)

    wstat = np.zeros((N_CHUNKS, 2, 128), dtype=np.float32)
    wmov = np.zeros((N_CHUNKS, 2, 512), dtype=np.float32)
    for ci, (si, tb) in enumerate(CHUNKS):
        d2 = 4 * tb - si
        # W[i,j] = w0[i]*d0^(j-i) + w1[j]*d1^(j-i), j-i = 128*d2 + jj - ii
        wstat[ci, 0] = (w0[128 * si:128 * (si + 1)] * d0 ** (-ii / DC)
                        ).astype(np.float32)
        wstat[ci, 1] = (d1 ** ((128 * d2 - ii) / DC)).astype(np.float32)
        wmov[ci, 0] = (d0 ** ((128 * d2 + jj) / DC)).astype(np.float32)
        wmov[ci, 1] = (w1[512 * tb:512 * (tb + 1)] * d1 ** (jj / DC)
                       ).astype(np.float32)

    masks = np.zeros((4, 128, 512), dtype=np.float32)
    for m in range(4):
        d2 = m - 3
        masks[m] = (128 * d2 + jj[None, :] - ii[:, None] >= 0
                    ).astype(np.float32)

    biasT = np.ascontiguousarray(
        bias.astype(np.float32).reshape(ST, 128).T)
    return wstat, wmov, masks, biasT


def make_in_maps(inputs):
    x = np.asarray(inputs["x"], dtype=np.float32)
    weight = np.asarray(inputs["weight"], dtype=np.float32)
    bias = np.asarray(inputs["bias"], dtype=np.float32)
    decay_value = np.asarray(inputs["decay_value"], dtype=np.float32)
    wstat, wmov, masks, biasT = _host_prep(weight, bias, decay_value)
    x2 = x.reshape(B * E, S)
    in_maps = []
    for c in range(N_CORES):
        xT_c = np.ascontiguousarray(x2[R * c:R * (c + 1), :].T)
        in_maps.append({
            "xT": xT_c, "wstat": wstat, "wmov": wmov,
            "masks": masks, "biasT": biasT,
        })
    return in_maps


def kernel(x, weight, bias, decay_value, index=0, recurrent=0, **_):
    global _PROGRAM
    if _PROGRAM is None:
        _PROGRAM = _build_program()
    nc = _PROGRAM

    in_maps = make_in_maps({"x": x, "weight": weight, "bias": bias,
                            "decay_value": decay_value})

    res = run_bass_kernel_spmd(nc, in_maps, core_ids=list(range(N_CORES)))
    out = np.empty((B * E, S), dtype=np.float32)
    for c in range(N_CORES):
        out[R * c:R * (c + 1), :] = res.results[c]["outT"].T
    return out.reshape(B, E, S)



# revision 4
# speedup vs baseline: 2.3052x; 2.3052x over previous
"""Trainium2 Bass kernel for CombinedRepeatCausalLinear (parallel forward).

Computes out[b,e,t] = sum_s x[b,e,s] * W[s,t] + bias[t] where
  W[s,t] = mask(t>=s) * (w0[s]*d0^(t-s) + w1[t]*d1^(t-s))
for S = 2048, x of shape (8, 1024, 2048) fp32.

Strategy (8 NeuronCores, data-parallel over batch):
  W is causal-masked rank-2.  Split t into 16 chunks of 128.  For target
  chunk J, the contribution from all source chunks I < J is exactly rank-2
  per chunk:
     out[t in J] = Wdiag_J-part + v0[t]*A[2J] + v1[t]*A[2J+1]
  where A[2J,e]   = sum_{s < 128J} w0[s] d0^(128J-s) x[s,e]
        A[2J+1,e] = sum_{s < 128J} d1^(128J-s) x[s,e]
  A (32 x 1024 per core) is accumulated in PSUM by 15 matmuls with tiny
  host-precomputed stationaries U_I [128, 32]; the per-chunk cross term is
  a K=32 matmul with stationary V_J [32, 128]; the diagonal 128x128 block
  is generated on-chip (K=2 matmul from factor vectors + causal mask) and
  applied as a K=128 matmul.  This cuts PE row-count ~2.8x vs dense
  triangular matmul.  All SBUF data is fp16 (halves DMA and doubles DVE
  throughput); PSUM accumulation stays fp32.  measured rel_err ~4.5e-4.
"""

import numpy as np

import concourse.bass as bass
import concourse.mybir as mybir
import concourse.tile as tile
from concourse import bacc
from concourse.bass_utils import run_bass_kernel_spmd

F16 = mybir.dt.float16
F32 = mybir.dt.float32

B = 8
E = 1024
S = 2048
DC = 1.0
N_CORES = 8
R = (B * E) // N_CORES      # rows (e) per core = 1024
C = 128                     # chunk size along s/t
NCH = S // C                # 16 chunks
NG = 4                      # x load groups (4 chunks = 1 MiB each)

_PROGRAM = None


def _build_program():
    nc = bacc.Bacc("TRN2", target_bir_lowering=False, debug=False,
                   num_devices=N_CORES)

    xg_d = nc.declare_dram_parameter("xg", [C, NCH * R], F16, isOutput=False)
    uu_d = nc.declare_dram_parameter("uu", [C, (NCH - 1) * 32], F16,
                                     isOutput=False)
    vv_d = nc.declare_dram_parameter("vv", [32, (NCH - 1) * C], F16,
                                     isOutput=False)
    fs_d = nc.declare_dram_parameter("fs", [2, NCH * C], F16, isOutput=False)
    fm_d = nc.declare_dram_parameter("fm", [2, NCH * C], F16, isOutput=False)
    mask_d = nc.declare_dram_parameter("mask", [C, C], F16, isOutput=False)
    biasT_d = nc.declare_dram_parameter("biasT", [C, NCH], F32,
                                        isOutput=False)
    outg_d = nc.declare_dram_parameter("outg", [C, NCH * R], F16,
                                       isOutput=True)

    Ident = mybir.ActivationFunctionType.Identity

    with tile.TileContext(nc) as tc:
        with (
            tc.tile_pool(name="cst", bufs=1) as cst,
            tc.tile_pool(name="xp", bufs=1) as xp,
            tc.tile_pool(name="wd", bufs=NCH) as wdp,
            tc.tile_pool(name="osb", bufs=4) as osb,
            tc.tile_pool(name="pa", bufs=1, space="PSUM") as pap,
            tc.tile_pool(name="pw", bufs=2, space="PSUM") as pwp,
            tc.tile_pool(name="po", bufs=4, space="PSUM") as pop,
        ):
            # --- constant / parameter loads (gpsimd SWDGE queue) ---
            uu_sb = cst.tile([C, (NCH - 1) * 32], F16, tag="uu")
            nc.gpsimd.dma_start(uu_sb[:], uu_d[:])
            vv_sb = cst.tile([32, (NCH - 1) * C], F16, tag="vv")
            nc.gpsimd.dma_start(vv_sb[:], vv_d[:])
            fs_sb = cst.tile([2, NCH * C], F16, tag="fs")
            nc.gpsimd.dma_start(fs_sb[:], fs_d[:])
            fm_sb = cst.tile([2, NCH * C], F16, tag="fm")
            nc.gpsimd.dma_start(fm_sb[:], fm_d[:])
            mask_sb = cst.tile([C, C], F16, tag="mask")
            nc.gpsimd.dma_start(mask_sb[:], mask_d[:])
            bias_sb = cst.tile([C, NCH], F32, tag="bias")
            nc.gpsimd.dma_start(bias_sb[:], biasT_d[:])

            # --- x loads: 4 groups of 4 chunks (1 MiB each, sync HWDGE) ---
            xg = []
            for g in range(NG):
                t = xp.tile([C, 4 * R], F16, tag=f"xg{g}")
                nc.sync.dma_start(t[:], xg_d[:, 4 * R * g:4 * R * (g + 1)])
                xg.append(t)

            def xsl(I, h):
                """moving slice for chunk I, half h: [128, 512]"""
                g, l = I // 4, I % 4
                c0 = R * l + 512 * h
                return xg[g][:, c0:c0 + 512]

            # --- Wdiag generation: K=2 matmul + causal mask, all 16 up front
            wd_sb = []
            for J in range(NCH):
                pw = pwp.tile([C, C], F32, tag="pw")
                nc.tensor.matmul(pw[:], fs_sb[:, C * J:C * (J + 1)],
                                 fm_sb[:, C * J:C * (J + 1)],
                                 start=True, stop=True)
                w = wdp.tile([C, C], F16, tag="wd", name=f"wd{J}")
                nc.vector.tensor_mul(w[:], pw[:], mask_sb[:])
                wd_sb.append(w)

            # --- A-phase: accumulate cross-chunk state [32, 1024] in PSUM ---
            a_ps = [pap.tile([32, 512], F32, tag=f"pa{h}", name=f"pa{h}")
                    for h in range(2)]
            for I in range(NCH - 1):
                for h in range(2):
                    nc.tensor.matmul(a_ps[h][:],
                                     uu_sb[:, 32 * I:32 * (I + 1)],
                                     xsl(I, h),
                                     start=(I == 0), stop=(I == NCH - 2))
            a_sb = cst.tile([32, R], F16, tag="a")
            nc.scalar.activation(a_sb[:, 0:512], a_ps[0][:], Ident)
            nc.vector.tensor_copy(a_sb[:, 512:1024], a_ps[1][:])

            # --- main: per chunk J: diag matmul + rank-2 cross matmul ---
            for J in range(NCH):
                po = [pop.tile([C, 512], F32, tag="po", name=f"po{J}_{h}")
                      for h in range(2)]
                out_sb = osb.tile([C, R], F16, tag="osb")
                for h in range(2):
                    nc.tensor.matmul(po[h][:], wd_sb[J][:], xsl(J, h),
                                     start=True, stop=(J == 0))
                    if J > 0:
                        nc.tensor.matmul(po[h][:],
                                         vv_sb[:, C * (J - 1):C * J],
                                         a_sb[:, 512 * h:512 * (h + 1)],
                                         start=False, stop=True)
                nc.scalar.activation(out_sb[:, 0:512], po[0][:], Ident,
                                     bias=bias_sb[:, J:J + 1])
                nc.vector.tensor_scalar_add(out_sb[:, 512:1024], po[1][:],
                                            bias_sb[:, J:J + 1])
                nc.sync.dma_start(outg_d[:, R * J:R * (J + 1)], out_sb[:])

    nc.compile()
    return nc


def _host_prep(weight, bias, decay_value):
    w0 = weight[0].astype(np.float64)
    w1 = weight[1].astype(np.float64)
    d0 = float(np.clip(np.float32(decay_value[0, 0]), 0.9, 1.0))
    d1 = float(np.clip(np.float32(decay_value[1, 0]), 0.9, 1.0))
    sl = np.arange(C, dtype=np.float64)

    uu = np.zeros((C, (NCH - 1) * 32), dtype=np.float16)
    with np.errstate(under='ignore'):
        for I in range(NCH - 1):
            for J in range(I + 1, NCH):
                e = (128.0 * (J - I) - sl) / DC
                uu[:, 32 * I + 2 * J] = (w0[C * I:C * (I + 1)] * d0 ** e
                                         ).astype(np.float16)
                uu[:, 32 * I + 2 * J + 1] = (d1 ** e).astype(np.float16)

        vv = np.zeros((32, (NCH - 1) * C), dtype=np.float16)
        for J in range(1, NCH):
            c0 = C * (J - 1)
            vv[2 * J, c0:c0 + C] = (d0 ** (sl / DC)).astype(np.float16)
            vv[2 * J + 1, c0:c0 + C] = (w1[C * J:C * (J + 1)]
                                        * d1 ** (sl / DC)).astype(np.float16)

        fs = np.zeros((2, NCH * C), dtype=np.float16)
        fm = np.zeros((2, NCH * C), dtype=np.float16)
        for J in range(NCH):
            c0 = C * J
            fs[0, c0:c0 + C] = (w0[c0:c0 + C] * d0 ** ((64.0 - sl) / DC)
                                ).astype(np.float16)
            fm[0, c0:c0 + C] = (d0 ** ((sl - 64.0) / DC)).astype(np.float16)
            fs[1, c0:c0 + C] = (d1 ** ((64.0 - sl) / DC)).astype(np.float16)
            fm[1, c0:c0 + C] = (w1[c0:c0 + C] * d1 ** ((sl - 64.0) / DC)
                                ).astype(np.float16)

    mask = (sl[None, :] >= sl[:, None]).astype(np.float16)
    biasT = np.ascontiguousarray(
        bias.astype(np.float32).reshape(NCH, C).T)
    return uu, vv, fs, fm, mask, biasT


def make_in_maps(inputs):
    x = np.asarray(inputs["x"], dtype=np.float32)
    weight = np.asarray(inputs["weight"], dtype=np.float32)
    bias = np.asarray(inputs["bias"], dtype=np.float32)
    decay_value = np.asarray(inputs["decay_value"], dtype=np.float32)

    uu, vv, fs, fm, mask, biasT = _host_prep(weight, bias, decay_value)

    x16 = x.reshape(B * E, S).astype(np.float16)
    in_maps = []
    for c in range(N_CORES):
        # xg[p, R*I + j] = x16[R*c + j, 128*I + p]
        xc = x16[R * c:R * (c + 1), :]                     # [R, S]
        xgc = np.ascontiguousarray(
            xc.T.reshape(NCH, C, R).transpose(1, 0, 2).reshape(C, NCH * R))
        in_maps.append({
            "xg": xgc, "uu": uu, "vv": vv, "fs": fs, "fm": fm,
            "mask": mask, "biasT": biasT,
        })
    return in_maps


def kernel(x, weight, bias, decay_value, index=0, recurrent=0, **_):
    global _PROGRAM
    if _PROGRAM is None:
        _PROGRAM = _build_program()
    nc = _PROGRAM

    in_maps = make_in_maps({"x": x, "weight": weight, "bias": bias,
                            "decay_value": decay_value})

    res = run_bass_kernel_spmd(nc, in_maps, core_ids=list(range(N_CORES)))
    out = np.empty((B * E, S), dtype=np.float32)
    for c in range(N_CORES):
        og = np.asarray(res.results[c]["outg"])            # [C, NCH*R] f16
        # out[R*c + j, 128*J + p] = og[p, R*J + j]
        ot = og.reshape(C, NCH, R).transpose(1, 0, 2).reshape(S, R)
        out[R * c:R * (c + 1), :] = ot.T.astype(np.float32)
    return out.reshape(B, E, S)


# revision 8
# speedup vs baseline: 2.4947x; 1.0822x over previous
"""Trainium2 Bass kernel for CombinedRepeatCausalLinear (parallel forward).

Computes out[b,e,t] = sum_s x[b,e,s] * W[s,t] + bias[t] where
  W[s,t] = mask(t>=s) * (w0[s]*d0^(t-s) + w1[t]*d1^(t-s))
for S = 2048, x of shape (8, 1024, 2048) fp32.

Strategy (8 NeuronCores, data-parallel over batch; fp16 datapath):
  W is causal-masked rank-2.  Split s/t into 17 chunks of C=126.  For
  target chunk J the contribution of all s < 126J is exactly rank 2:
     out[t in J] = (diag block) + d0^tl * A0_J + w1[t] d1^tl * A1_J
  with A0_J[e] = sum_{s<126J} w0[s] d0^(126J-s) x[s,e]  (A1 analogous).
  C=126 leaves 2 spare K-rows, so the cross term folds into the SAME
  K=128 matmul as the 126x126 diagonal block: moving-operand partitions
  0/1 carry the per-chunk A rows (scattered in via SBUF->SBUF DMA,
  which has no partition-alignment restriction), partitions 2..127
  carry the x chunk; the stationary's rows 0/1 are the decay rows,
  generated together with the diag block by one K=2 matmul + mask.
  One K=128 matmul per (chunk, 512-half) covers the output (~17.4k PE
  rows).  A itself is accumulated by 16 col-tiled (tile_position)
  matmuls into 4 32-partition PSUM strips and summed by one
  ones-stationary matmul.  fp16 everywhere in SBUF (halves DMA);
  fp32 PSUM accumulate.  Measured rel_err ~4.5e-4.
"""

import numpy as np

import concourse.bass as bass
import concourse.mybir as mybir
import concourse.tile as tile
from concourse import bacc
from concourse.bass_utils import run_bass_kernel_spmd

F16 = mybir.dt.float16
F32 = mybir.dt.float32

B = 8
E = 1024
S = 2048
DC = 1.0
N_CORES = 8
R = (B * E) // N_CORES      # rows (e) per core = 1024
C = 126                     # chunk size along s/t
NCH = 17                    # chunks; chunk 16 has only 32 valid rows
LAST = S - C * (NCH - 1)    # 32

_PROGRAM = None


def _build_program():
    nc = bacc.Bacc("TRN2", target_bir_lowering=False, debug=False,
                   num_devices=N_CORES)

    xg_d = nc.declare_dram_parameter("xg", [C, NCH * R], F16, isOutput=False)
    uu_d = nc.declare_dram_parameter("uu", [128, 16 * 32], F16,
                                     isOutput=False)
    fs_d = nc.declare_dram_parameter("fs", [2, NCH * 128], F16,
                                     isOutput=False)
    fm_d = nc.declare_dram_parameter("fm", [2, NCH * C], F16, isOutput=False)
    mask_d = nc.declare_dram_parameter("mask", [128, C], F16, isOutput=False)
    pred_d = nc.declare_dram_parameter("pred", [128, 32], F16, isOutput=False)
    biasT_d = nc.declare_dram_parameter("biasT", [C, NCH], F32,
                                        isOutput=False)
    outg_d = nc.declare_dram_parameter("outg", [C, NCH * R], F16,
                                       isOutput=True)

    Ident = mybir.ActivationFunctionType.Identity

    with tile.TileContext(nc) as tc:
        with (
            tc.tile_pool(name="cst", bufs=1) as cst,
            tc.tile_pool(name="xp", bufs=1) as xp,
            tc.tile_pool(name="wd", bufs=NCH) as wdp,
            tc.tile_pool(name="osb", bufs=4) as osb,
            tc.tile_pool(name="pa", bufs=1, space="PSUM") as pap,
            tc.tile_pool(name="pw", bufs=2, space="PSUM") as pwp,
            tc.tile_pool(name="po", bufs=2, space="PSUM") as pop,
        ):
            # --- parameter loads (sync HWDGE ring, first) ---
            uu_sb = cst.tile([128, 16 * 32], F16, tag="uu")
            nc.sync.dma_start(uu_sb[:], uu_d[:])
            fs_sb = cst.tile([2, NCH * 128], F16, tag="fs")
            nc.sync.dma_start(fs_sb[:], fs_d[:])
            fm_sb = cst.tile([2, NCH * C], F16, tag="fm")
            nc.sync.dma_start(fm_sb[:], fm_d[:])
            mask_sb = cst.tile([128, C], F16, tag="mask")
            nc.sync.dma_start(mask_sb[:], mask_d[:])
            pred_sb = cst.tile([128, 32], F16, tag="pred")
            nc.sync.dma_start(pred_sb[:], pred_d[:])
            bias_sb = cst.tile([C, NCH], F32, tag="bias")
            nc.sync.dma_start(bias_sb[:], biasT_d[:])

            # --- x loads into rows 2..127; rows 0/1 are per-chunk A rows ---
            xg = xp.tile([128, NCH * R], F16, tag="xg")
            nc.gpsimd.memset(xg[0:2, :], 0.0)
            nc.sync.dma_start(xg[2:128, 0:4 * R], xg_d[:, 0:4 * R])
            nc.sync.dma_start(xg[2:128, 4 * R:8 * R], xg_d[:, 4 * R:8 * R])
            nc.scalar.dma_start(xg[2:128, 8 * R:12 * R],
                                xg_d[:, 8 * R:12 * R])
            nc.scalar.dma_start(xg[2:128, 12 * R:NCH * R],
                                xg_d[:, 12 * R:NCH * R])

            # --- stationary generation: rank-2 with embedded decay rows ---
            wf_sb = []
            for J in range(NCH):
                pw = pwp.tile([128, C], F32, tag="pw")
                nc.tensor.matmul(pw[:], fs_sb[:, 128 * J:128 * (J + 1)],
                                 fm_sb[:, C * J:C * (J + 1)],
                                 start=True, stop=True)
                wf = wdp.tile([128, C], F16, tag="wd", name=f"wd{J}")
                nc.vector.tensor_mul(wf[:], pw[:], mask_sb[:])
                wf_sb.append(wf)

            # --- A-phase: 16 col-tiled matmuls into 4 PSUM strips ---
            a_ps = [pap.tile([128, 512], F32, tag=f"pa{h}", name=f"pa{h}")
                    for h in range(2)]
            for I in range(NCH - 1):
                g = I % 4
                for h in range(2):
                    nc.tensor.matmul(a_ps[h][32 * g:32 * (g + 1), :],
                                     uu_sb[:, 32 * I:32 * (I + 1)],
                                     xg[:, R * I + 512 * h:
                                        R * I + 512 * (h + 1)],
                                     start=(I <= 3), stop=(I >= NCH - 5),
                                     skip_group_check=True,
                                     tile_position=(0, 32 * g))
            a4_sb = cst.tile([128, R], F16, tag="a4")
            nc.scalar.activation(a4_sb[:, 0:512], a_ps[0][:], Ident)
            nc.vector.tensor_copy(a4_sb[:, 512:1024], a_ps[1][:])

            # --- reduce 4 strips: ones-stationary matmul (reuses pa banks)
            ar_ps = [pap.tile([32, 512], F32, tag=f"pa{h}", name=f"ar{h}")
                     for h in range(2)]
            for h in range(2):
                nc.tensor.matmul(ar_ps[h][:], pred_sb[:],
                                 a4_sb[:, 512 * h:512 * (h + 1)],
                                 start=True, stop=True)
            a2_sb = cst.tile([32, R], F16, tag="a2")
            nc.scalar.activation(a2_sb[:, 0:512], ar_ps[0][:], Ident)
            nc.vector.tensor_copy(a2_sb[:, 512:1024], ar_ps[1][:])

            # --- scatter A rows into chunk slots (SBUF->SBUF DMA) ---
            for J in range(1, NCH):
                nc.gpsimd.dma_start(xg[0:2, R * J:R * (J + 1)],
                                    a2_sb[2 * (J - 1):2 * J, :])

            # --- main: one K=128 matmul per (chunk, half) + bias copy ---
            for J in range(NCH):
                po = pop.tile([C, R], F32, tag="po", name=f"po{J}")
                out_sb = osb.tile([C, R], F16, tag="osb")
                for h in range(2):
                    nc.tensor.matmul(po[:, 512 * h:512 * (h + 1)], wf_sb[J][:],
                                     xg[:, R * J + 512 * h:
                                        R * J + 512 * (h + 1)],
                                     start=True, stop=True)
                if J % 2 == 0:
                    nc.scalar.activation(out_sb[:], po[:], Ident,
                                         bias=bias_sb[:, J:J + 1])
                    nc.sync.dma_start(outg_d[:, R * J:R * (J + 1)], out_sb[:])
                else:
                    nc.vector.tensor_scalar_add(out_sb[:], po[:],
                                                bias_sb[:, J:J + 1])
                    nc.scalar.dma_start(outg_d[:, R * J:R * (J + 1)],
                                        out_sb[:])

    nc.compile()
    return nc


def _host_prep(weight, bias, decay_value):
    w0 = np.zeros(C * NCH); w1 = np.zeros(C * NCH)
    w0[:S] = weight[0].astype(np.float64)
    w1[:S] = weight[1].astype(np.float64)
    d0 = float(np.clip(np.float32(decay_value[0, 0]), 0.9, 1.0))
    d1 = float(np.clip(np.float32(decay_value[1, 0]), 0.9, 1.0))
    sl = np.arange(C, dtype=np.float64)

    uu = np.zeros((128, 16 * 32), dtype=np.float16)
    fs = np.zeros((2, NCH * 128), dtype=np.float16)
    fm = np.zeros((2, NCH * C), dtype=np.float16)
    with np.errstate(under='ignore'):
        for I in range(NCH - 1):
            for J in range(I + 1, NCH):
                e = (126.0 * (J - I) - sl) / DC
                m = 2 * (J - 1)
                uu[2:128, 32 * I + m] = (w0[C * I:C * (I + 1)] * d0 ** e
                                         ).astype(np.float16)
                uu[2:128, 32 * I + m + 1] = (d1 ** e).astype(np.float16)
        for J in range(NCH):
            c0 = C * J
            # stationary factor rows: [p=0] decay row v0, [p=1] v1,
            # [p>=2] diag-block factors (s_loc = p-2)
            if J > 0:
                fs[0, 128 * J + 0] = np.float16(d0 ** (63.0 / DC))
                fs[1, 128 * J + 1] = np.float16(d1 ** (63.0 / DC))
            fs[0, 128 * J + 2:128 * (J + 1)] = (
                w0[c0:c0 + C] * d0 ** ((63.0 - sl) / DC)).astype(np.float16)
            fs[1, 128 * J + 2:128 * (J + 1)] = (
                d1 ** ((63.0 - sl) / DC)).astype(np.float16)
            fm[0, c0:c0 + C] = (d0 ** ((sl - 63.0) / DC)).astype(np.float16)
            fm[1, c0:c0 + C] = (w1[c0:c0 + C] * d1 ** ((sl - 63.0) / DC)
                                ).astype(np.float16)
        fm[:, C * 16 + LAST:] = 0

    mask = np.zeros((128, C), dtype=np.float16)
    mask[0:2, :] = 1
    mask[2:128, :] = (sl[None, :] >= sl[:, None]).astype(np.float16)
    pred = np.zeros((128, 32), dtype=np.float16)
    for g in range(4):
        pred[32 * g:32 * (g + 1)] = np.eye(32, dtype=np.float16)
    biasT = np.zeros((C, NCH), dtype=np.float32)
    bias32 = bias.astype(np.float32)
    for J in range(NCH):
        hi = min(C, S - C * J)
        biasT[:hi, J] = bias32[C * J:C * J + hi]
    return uu, fs, fm, mask, pred, biasT


def make_in_maps(inputs):
    x = np.asarray(inputs["x"], dtype=np.float32)
    weight = np.asarray(inputs["weight"], dtype=np.float32)
    bias = np.asarray(inputs["bias"], dtype=np.float32)
    decay_value = np.asarray(inputs["decay_value"], dtype=np.float32)

    uu, fs, fm, mask, pred, biasT = _host_prep(weight, bias, decay_value)

    x16 = x.reshape(B * E, S).astype(np.float16)
    in_maps = []
    for c in range(N_CORES):
        xc = x16[R * c:R * (c + 1), :]                    # [R, S]
        xgc = np.zeros((C, NCH * R), dtype=np.float16)
        xcT = xc.T                                        # [S, R]
        for J in range(NCH):
            hi = min(C, S - C * J)
            xgc[0:hi, R * J:R * (J + 1)] = xcT[C * J:C * J + hi, :]
        in_maps.append({
            "xg": np.ascontiguousarray(xgc), "uu": uu,
            "fs": fs, "fm": fm, "mask": mask, "pred": pred, "biasT": biasT,
        })
    return in_maps


def kernel(x, weight, bias, decay_value, index=0, recurrent=0, **_):
    global _PROGRAM
    if _PROGRAM is None:
        _PROGRAM = _build_program()
    nc = _PROGRAM

    in_maps = make_in_maps({"x": x, "weight": weight, "bias": bias,
                            "decay_value": decay_value})

    res = run_bass_kernel_spmd(nc, in_maps, core_ids=list(range(N_CORES)))
    out = np.empty((B * E, S), dtype=np.float32)
    for c in range(N_CORES):
        og = np.asarray(res.results[c]["outg"])            # [C, NCH*R] f16
        ot = np.empty((S, R), dtype=np.float32)
        for J in range(NCH):
            hi = min(C, S - C * J)
            ot[C * J:C * J + hi, :] = og[0:hi, R * J:R * (J + 1)
                                         ].astype(np.float32)
        out[R * c:R * (c + 1), :] = ot.T
    return out.reshape(B, E, S)


# revision 11
# speedup vs baseline: 2.6328x; 1.0554x over previous
"""Trainium2 Bass kernel for CombinedRepeatCausalLinear (parallel forward).

Computes out[b,e,t] = sum_s x[b,e,s] * W[s,t] + bias[t] where
  W[s,t] = mask(t>=s) * (w0[s]*d0^(t-s) + w1[t]*d1^(t-s))
for S = 2048, x of shape (8, 1024, 2048) fp32.

Strategy (8 NeuronCores, data-parallel over batch; fp16 datapath):
  W is causal-masked rank-2.  Split s/t into 17 chunks of C=126.  For
  target chunk J the contribution of all s < 126J is exactly rank 2:
     out[t in J] = (diag block) + d0^tl * A0_J + w1[t] d1^tl * A1_J
  with A0_J[e] = sum_{s<126J} w0[s] d0^(126J-s) x[s,e]  (A1 analogous).
  C=126 leaves 2 spare K-rows, so the cross term folds into the SAME
  K=128 matmul as the 126x126 diagonal block: moving-operand partitions
  0/1 carry the per-chunk A rows (scattered in via SBUF->SBUF DMA,
  which has no partition-alignment restriction), partitions 2..127
  carry the x chunk; the stationary's rows 0/1 are the decay rows,
  generated together with the diag block by one K=2 matmul + mask.
  One K=128 matmul per (chunk, 512-half) covers the output (~17.4k PE
  rows).  A itself is accumulated by 16 col-tiled (tile_position)
  matmuls into 4 32-partition PSUM strips and summed by one
  ones-stationary matmul.  fp16 everywhere in SBUF (halves DMA);
  fp32 PSUM accumulate.  Measured rel_err ~4.5e-4.
"""

import numpy as np

import concourse.bass as bass
import concourse.mybir as mybir
import concourse.tile as tile
from concourse import bacc
from concourse.bass_utils import run_bass_kernel_spmd

F16 = mybir.dt.float16
F32 = mybir.dt.float32

B = 8
E = 1024
S = 2048
DC = 1.0
N_CORES = 8
R = (B * E) // N_CORES      # rows (e) per core = 1024
C = 126                     # chunk size along s/t
NCH = 17                    # chunks; chunk 16 has only 32 valid rows
LAST = S - C * (NCH - 1)    # 32

_PROGRAM = None


def _build_program():
    nc = bacc.Bacc("TRN2", target_bir_lowering=False, debug=False,
                   num_devices=N_CORES)

    xg_d = nc.declare_dram_parameter("xg", [C, NCH * R], F16, isOutput=False)
    uu_d = nc.declare_dram_parameter("uu", [128, 16 * 32], F16,
                                     isOutput=False)
    fs_d = nc.declare_dram_parameter("fs", [2, NCH * 128], F16,
                                     isOutput=False)
    fm_d = nc.declare_dram_parameter("fm", [2, NCH * C], F16, isOutput=False)
    mask_d = nc.declare_dram_parameter("mask", [128, C], F16, isOutput=False)
    pred_d = nc.declare_dram_parameter("pred", [128, 32], F16, isOutput=False)
    biasT_d = nc.declare_dram_parameter("biasT", [C, NCH], F32,
                                        isOutput=False)
    outg_d = nc.declare_dram_parameter("outg", [C, NCH * R], F16,
                                       isOutput=True)

    Ident = mybir.ActivationFunctionType.Identity

    with tile.TileContext(nc) as tc:
        with (
            tc.tile_pool(name="cst", bufs=1) as cst,
            tc.tile_pool(name="xp", bufs=1) as xp,
            tc.tile_pool(name="wd", bufs=NCH) as wdp,
            tc.tile_pool(name="osb", bufs=4) as osb,
            tc.tile_pool(name="pa", bufs=1, space="PSUM") as pap,
            tc.tile_pool(name="pw", bufs=2, space="PSUM") as pwp,
            tc.tile_pool(name="po", bufs=2, space="PSUM") as pop,
        ):
            # --- parameter loads (scalar HWDGE ring, first) ---
            uu_sb = cst.tile([128, 16 * 32], F16, tag="uu")
            nc.scalar.dma_start(uu_sb[:], uu_d[:])
            fs_sb = cst.tile([2, NCH * 128], F16, tag="fs")
            nc.scalar.dma_start(fs_sb[:], fs_d[:])
            fm_sb = cst.tile([2, NCH * C], F16, tag="fm")
            nc.scalar.dma_start(fm_sb[:], fm_d[:])
            mask_sb = cst.tile([128, C], F16, tag="mask")
            nc.scalar.dma_start(mask_sb[:], mask_d[:])
            pred_sb = cst.tile([128, 32], F16, tag="pred")
            nc.scalar.dma_start(pred_sb[:], pred_d[:])
            bias_sb = cst.tile([C, NCH], F32, tag="bias")
            nc.scalar.dma_start(bias_sb[:], biasT_d[:])

            # --- x loads into rows 2..127; rows 0/1 are per-chunk A rows ---
            xg = xp.tile([128, NCH * R], F16, tag="xg")
            nc.gpsimd.memset(xg[0:2, :], 0.0)
            nc.sync.dma_start(xg[2:128, 0:4 * R], xg_d[:, 0:4 * R])
            nc.sync.dma_start(xg[2:128, 4 * R:8 * R], xg_d[:, 4 * R:8 * R])
            nc.scalar.dma_start(xg[2:128, 8 * R:12 * R],
                                xg_d[:, 8 * R:12 * R])
            nc.scalar.dma_start(xg[2:128, 12 * R:NCH * R],
                                xg_d[:, 12 * R:NCH * R])

            # --- stationary generation: rank-2 with embedded decay rows ---
            wf_sb = []
            for J in range(NCH):
                pw = pwp.tile([128, C], F32, tag="pw")
                nc.tensor.matmul(pw[:], fs_sb[:, 128 * J:128 * (J + 1)],
                                 fm_sb[:, C * J:C * (J + 1)],
                                 start=True, stop=True)
                wf = wdp.tile([128, C], F16, tag="wd", name=f"wd{J}")
                nc.vector.tensor_mul(wf[:], pw[:], mask_sb[:])
                wf_sb.append(wf)

            # --- A-phase: 16 col-tiled matmuls into 4 PSUM strips ---
            a_ps = [pap.tile([128, 512], F32, tag=f"pa{h}", name=f"pa{h}")
                    for h in range(2)]
            for I in range(NCH - 1):
                g = I % 4
                for h in range(2):
                    nc.tensor.matmul(a_ps[h][32 * g:32 * (g + 1), :],
                                     uu_sb[:, 32 * I:32 * (I + 1)],
                                     xg[:, R * I + 512 * h:
                                        R * I + 512 * (h + 1)],
                                     start=(I <= 3), stop=(I >= NCH - 5),
                                     skip_group_check=True,
                                     tile_position=(0, 32 * g))
            a4_sb = cst.tile([128, R], F16, tag="a4")
            nc.scalar.activation(a4_sb[:, 0:512], a_ps[0][:], Ident)
            nc.vector.tensor_copy(a4_sb[:, 512:1024], a_ps[1][:])

            # --- reduce 4 strips: ones-stationary matmul (reuses pa banks)
            ar_ps = [pap.tile([32, 512], F32, tag=f"pa{h}", name=f"ar{h}")
                     for h in range(2)]
            for h in range(2):
                nc.tensor.matmul(ar_ps[h][:], pred_sb[:],
                                 a4_sb[:, 512 * h:512 * (h + 1)],
                                 start=True, stop=True)
            a2_sb = cst.tile([32, R], F16, tag="a2")
            nc.scalar.activation(a2_sb[:, 0:512], ar_ps[0][:], Ident)
            nc.vector.tensor_copy(a2_sb[:, 512:1024], ar_ps[1][:])

            # --- scatter A rows into chunk slots (SBUF->SBUF DMA, 4 queues)
            qs = [nc.sync, nc.scalar, nc.gpsimd]
            for J in range(1, NCH):
                qs[(J - 1) % 3].dma_start(xg[0:2, R * J:R * (J + 1)],
                                          a2_sb[2 * (J - 1):2 * J, :])

            # --- main: one K=128 matmul per (chunk, half) + bias copy ---
            for J in range(NCH):
                po = pop.tile([C, R], F32, tag="po", name=f"po{J}")
                out_sb = osb.tile([C, R], F16, tag="osb")
                for h in range(2):
                    nc.tensor.matmul(po[:, 512 * h:512 * (h + 1)], wf_sb[J][:],
                                     xg[:, R * J + 512 * h:
                                        R * J + 512 * (h + 1)],
                                     start=True, stop=True)
                nc.scalar.activation(out_sb[:, 0:512], po[:, 0:512], Ident,
                                     bias=bias_sb[:, J:J + 1])
                nc.vector.tensor_scalar_add(out_sb[:, 512:1024],
                                            po[:, 512:1024],
                                            bias_sb[:, J:J + 1])
                if J % 2 == 0:
                    nc.sync.dma_start(outg_d[:, R * J:R * (J + 1)], out_sb[:])
                else:
                    nc.scalar.dma_start(outg_d[:, R * J:R * (J + 1)],
                                        out_sb[:])

    nc.compile()
    return nc


def _host_prep(weight, bias, decay_value):
    w0 = np.zeros(C * NCH); w1 = np.zeros(C * NCH)
    w0[:S] = weight[0].astype(np.float64)
    w1[:S] = weight[1].astype(np.float64)
    d0 = float(np.clip(np.float32(decay_value[0, 0]), 0.9, 1.0))
    d1 = float(np.clip(np.float32(decay_value[1, 0]), 0.9, 1.0))
    sl = np.arange(C, dtype=np.float64)

    uu = np.zeros((128, 16 * 32), dtype=np.float16)
    fs = np.zeros((2, NCH * 128), dtype=np.float16)
    fm = np.zeros((2, NCH * C), dtype=np.float16)
    with np.errstate(under='ignore'):
        for I in range(NCH - 1):
            for J in range(I + 1, NCH):
                e = (126.0 * (J - I) - sl) / DC
                m = 2 * (J - 1)
                uu[2:128, 32 * I + m] = (w0[C * I:C * (I + 1)] * d0 ** e
                                         ).astype(np.float16)
                uu[2:128, 32 * I + m + 1] = (d1 ** e).astype(np.float16)
        for J in range(NCH):
            c0 = C * J
            # stationary factor rows: [p=0] decay row v0, [p=1] v1,
            # [p>=2] diag-block factors (s_loc = p-2)
            if J > 0:
                fs[0, 128 * J + 0] = np.float16(d0 ** (63.0 / DC))
                fs[1, 128 * J + 1] = np.float16(d1 ** (63.0 / DC))
            fs[0, 128 * J + 2:128 * (J + 1)] = (
                w0[c0:c0 + C] * d0 ** ((63.0 - sl) / DC)).astype(np.float16)
            fs[1, 128 * J + 2:128 * (J + 1)] = (
                d1 ** ((63.0 - sl) / DC)).astype(np.float16)
            fm[0, c0:c0 + C] = (d0 ** ((sl - 63.0) / DC)).astype(np.float16)
            fm[1, c0:c0 + C] = (w1[c0:c0 + C] * d1 ** ((sl - 63.0) / DC)
                                ).astype(np.float16)
        fm[:, C * 16 + LAST:] = 0

    mask = np.zeros((128, C), dtype=np.float16)
    mask[0:2, :] = 1
    mask[2:128, :] = (sl[None, :] >= sl[:, None]).astype(np.float16)
    pred = np.zeros((128, 32), dtype=np.float16)
    for g in range(4):
        pred[32 * g:32 * (g + 1)] = np.eye(32, dtype=np.float16)
    biasT = np.zeros((C, NCH), dtype=np.float32)
    bias32 = bias.astype(np.float32)
    for J in range(NCH):
        hi = min(C, S - C * J)
        biasT[:hi, J] = bias32[C * J:C * J + hi]
    return uu, fs, fm, mask, pred, biasT


def make_in_maps(inputs):
    x = np.asarray(inputs["x"], dtype=np.float32)
    weight = np.asarray(inputs["weight"], dtype=np.float32)
    bias = np.asarray(inputs["bias"], dtype=np.float32)
    decay_value = np.asarray(inputs["decay_value"], dtype=np.float32)

    uu, fs, fm, mask, pred, biasT = _host_prep(weight, bias, decay_value)

    x16 = x.reshape(B * E, S).astype(np.float16)
    in_maps = []
    for c in range(N_CORES):
        xc = x16[R * c:R * (c + 1), :]                    # [R, S]
        xgc = np.zeros((C, NCH * R), dtype=np.float16)
        xcT = xc.T                                        # [S, R]
        for J in range(NCH):
            hi = min(C, S - C * J)
            xgc[0:hi, R * J:R * (J + 1)] = xcT[C * J:C * J + hi, :]
        in_maps.append({
            "xg": np.ascontiguousarray(xgc), "uu": uu,
            "fs": fs, "fm": fm, "mask": mask, "pred": pred, "biasT": biasT,
        })
    return in_maps


def kernel(x, weight, bias, decay_value, index=0, recurrent=0, **_):
    global _PROGRAM
    if _PROGRAM is None:
        _PROGRAM = _build_program()
    nc = _PROGRAM

    in_maps = make_in_maps({"x": x, "weight": weight, "bias": bias,
                            "decay_value": decay_value})

    res = run_bass_kernel_spmd(nc, in_maps, core_ids=list(range(N_CORES)))
    out = np.empty((B * E, S), dtype=np.float32)
    for c in range(N_CORES):
        og = np.asarray(res.results[c]["outg"])            # [C, NCH*R] f16
        ot = np.empty((S, R), dtype=np.float32)
        for J in range(NCH):
            hi = min(C, S - C * J)
            ot[C * J:C * J + hi, :] = og[0:hi, R * J:R * (J + 1)
                                         ].astype(np.float32)
        out[R * c:R * (c + 1), :] = ot.T
    return out.reshape(B, E, S)
